# revision 1
# baseline (speedup 1.0000x reference)
"""GPTQ 4-bit quant linear (nn_Autograd4bitQuantLinear) on 8 TRN2 NeuronCores.

Strategy (column-parallel tensor parallelism, per sharding hint):
 - Host: dequantize packed 4-bit weights to W [4096, 11008] f32, round to
   fp16, shard along out_features (1376 per core). Transpose x to
   xT [4096, 8192] fp16 (contraction dim on partitions), replicated.
 - Device (per core): xT.T @ W_shard on the PE in fp16 (same PE rate as
   bf16, 8x less rounding error; fp32 PSUM accumulation). W shard (11.3MB)
   stays resident in SBUF; x streams in 512-token blocks; psum chunks of
   512/512/352 out-features; out [8192, 1376] f32 written back.
 - Host: concatenate the 8 shards along the last dim.

Measured (r1/r8 repeat-differencing, pipelined): 1.44 ms/core sustained,
rel err 2.9e-4. PE roofline is 1.17 ms; single-core run measures 1.18 ms —
the sustained gap is chip-level power throttling under 4+ busy cores, not
scheduling (matmul-only variant times identical to the full kernel).
"""

import os
import numpy as np
import ml_dtypes

IN_F = 4096
OUT_F = 11008
TOKENS = 8192
NCORES = 8
SHARD = OUT_F // NCORES  # 1376
P = 128
KT = IN_F // P  # 32 k-tiles
TB = 512  # tokens per block
NBLK = TOKENS // TB  # 16
TSUB = TB // P  # 4
CHUNKS = [(0, 512), (512, 512), (1024, SHARD - 1024)]  # psum-bank sized out chunks
MM_DT = "float16"  # PE dtype for x and W: float16 (same PE rate as bf16, 8x less rounding error)

_CACHE = {}


def _build_nc(
    reps=1, nop=False, no_xdma=False, no_mm=False, no_out=False, chunk_outer=False
):
    import concourse.bass as bass
    import concourse.mybir as mybir
    import concourse.tile as tile
    from concourse import bacc

    nc = bacc.Bacc(
        "TRN2",
        target_bir_lowering=False,
        debug=False,
        enable_asserts=False,
        num_devices=NCORES,
    )
    mdt = getattr(mybir.dt, MM_DT)
    f32 = mybir.dt.float32
    xt = nc.dram_tensor("xt", [IN_F, TOKENS], mdt, kind="ExternalInput").ap()
    w = nc.dram_tensor("w", [IN_F, SHARD], mdt, kind="ExternalInput").ap()
    out = nc.dram_tensor("out", [TOKENS, SHARD], f32, kind="ExternalOutput").ap()

    with tile.TileContext(nc) as tc:
        with (
            tc.tile_pool(name="wp", bufs=1) as wp,
            tc.tile_pool(name="xp", bufs=2) as xp,
            tc.tile_pool(name="op", bufs=2) as op,
            tc.tile_pool(name="pp", bufs=2, space=bass.MemorySpace.PSUM) as pp,
        ):
            if nop:
                o_sb = op.tile([P, SHARD], f32)
                nc.gpsimd.memset(o_sb[:], 0.0)
                for r in range(TOKENS // P):
                    nc.sync.dma_start(out[r * P : (r + 1) * P, :], o_sb[:])
                nc.compile()
                return nc
            w_sb = wp.tile([P, KT, SHARD], mdt)
            for k in range(KT):
                nc.sync.dma_start(w_sb[:, k, :], w[k * P : (k + 1) * P, :])
            for _rep in range(reps):
                for b in range(NBLK):
                    x_sb = xp.tile([P, KT, TB], mdt)
                    if not no_xdma:
                        for k in range(KT):
                            nc.sync.dma_start(
                                x_sb[:, k, :],
                                xt[k * P : (k + 1) * P, b * TB : (b + 1) * TB],
                            )
                    else:
                        nc.gpsimd.memset(x_sb[:, 0, :], 0.0)
                    for s in range(TSUB):
                        o_sb = (
                            op.tile([P, SHARD], f32, name="o_sb")
                            if not no_out
                            else None
                        )
                        pss = (
                            [
                                pp.tile([P, 512], f32, tag=f"ps{ci}", name=f"ps{ci}")
                                for ci in range(len(CHUNKS))
                            ]
                            if not no_mm
                            else None
                        )
                        if not no_mm:
                            if chunk_outer:
                                # 32 consecutive same-bank MMs per chunk: avoids
                                # per-MM PSUM-bank cycling (HAM oscillation).
                                for ci, (n0, nw) in enumerate(CHUNKS):
                                    for k in range(KT):
                                        nc.tensor.matmul(
                                            pss[ci][:, :nw],
                                            x_sb[:, k, s * P : (s + 1) * P],
                                            w_sb[:, k, n0 : n0 + nw],
                                            start=(k == 0),
                                            stop=(k == KT - 1),
                                        )
                            else:
                                for k in range(KT):
                                    lhsT = x_sb[:, k, s * P : (s + 1) * P]
                                    for ci, (n0, nw) in enumerate(CHUNKS):
                                        nc.tensor.matmul(
                                            pss[ci][:, :nw],
                                            lhsT,
                                            w_sb[:, k, n0 : n0 + nw],
                                            start=(k == 0),
                                            stop=(k == KT - 1),
                                        )
                        if not no_out:
                            if no_mm:
                                nc.gpsimd.memset(o_sb[:], 0.0)
                            else:
                                for ci, (n0, nw) in enumerate(CHUNKS):
                                    nc.vector.tensor_copy(
                                        o_sb[:, n0 : n0 + nw], pss[ci][:, :nw]
                                    )
                            r0 = b * TB + s * P
                            nc.sync.dma_start(out[r0 : r0 + P, :], o_sb[:])
    nc.compile()
    return nc


def _dequant_f32(qweight, scales, qzeros, g_idx):
    """GPTQ v2 dequant: W = s * (w4 - (z4 + 1)), [in_features, out_features] f32."""
    shifts = np.arange(8, dtype=np.uint32) * 4
    qw = np.ascontiguousarray(qweight).view(np.uint32)
    w4 = (
        ((qw[:, None, :] >> shifts[None, :, None]) & np.uint32(0xF))
        .reshape(-1, qweight.shape[1])
        .astype(np.float32)
    )
    qz = np.ascontiguousarray(qzeros).view(np.uint32)
    z4 = (
        ((qz[:, :, None] >> shifts[None, None, :]) & np.uint32(0xF)).reshape(
            qzeros.shape[0], -1
        )
        + np.uint32(1)
    ).astype(np.float32)
    return scales[g_idx] * (w4 - z4[g_idx])


def kernel(x, qweight, scales, qzeros, g_idx):
    # NTFF tracing is unavailable under this axon client (antenv.axon_hooks
    # missing); force it off so a stray BASS_TRACE doesn't crash the run.
    os.environ["BASS_NEVER_TRACE"] = "1"
    from concourse.bass_utils import run_bass_kernel_spmd

    x = np.asarray(x, dtype=np.float32)
    qweight = np.asarray(qweight, dtype=np.int32)
    scales = np.asarray(scales, dtype=np.float32)
    qzeros = np.asarray(qzeros, dtype=np.int32)
    g_idx = np.asarray(g_idx, dtype=np.int32)

    mdt = np.float16 if MM_DT == "float16" else ml_dtypes.bfloat16
    W = _dequant_f32(qweight, scales, qzeros, g_idx)
    xt = np.ascontiguousarray(x.reshape(-1, IN_F).astype(mdt).T)

    if "nc" not in _CACHE:
        _CACHE["nc"] = _build_nc()
    nc = _CACHE["nc"]

    in_maps = []
    for c in range(NCORES):
        wshard = np.ascontiguousarray(W[:, c * SHARD : (c + 1) * SHARD].astype(mdt))
        in_maps.append({"xt": xt, "w": wshard})

    trace = os.environ.get("GPTQ_TRACE", "0") == "1"
    res = run_bass_kernel_spmd(nc, in_maps, core_ids=list(range(NCORES)), trace=trace)
    _CACHE["last_results"] = res

    out = np.concatenate([res.results[c]["out"] for c in range(NCORES)], axis=1)
    return np.ascontiguousarray(out.reshape(x.shape[0], x.shape[1], OUT_F))

